# revision 1
# baseline (speedup 1.0000x reference)
"""CLIP attention (ShareKey branch) Trainium2 Bass kernel, 8-core SPMD.

Math: in the reference, attn = softmax(scores[..., None] + share_bias, axis=-1)
where scores is constant along the softmax axis -> softmax shift-invariance
makes the q-projection / share_key / scores irrelevant. The output is exactly

    P[h]   = softmax(share_bias[h], axis=-1)            (batch independent)
    V[b]   = hidden[b] @ v_w.T + v_b
    O[b,h] = P[h] @ V[b,h]                               (per-head slice of V)
    out[b] = concat_h(O[b,h]) @ out_w.T + out_b

Sharding: data-parallel over batch (16 batches / 8 cores = 2 per core);
weights + bias replicated per core. All transposes required to feed the PE
(contraction dim on partitions) are done host-side in numpy as part of input
layout: hiddenT = hidden^T per batch, wvT = v_w.T, woT = out_w.T, biasT =
share_bias^T per head. On-device, per core:

  V[j, (b,e)]     = sum_k hiddenT[b][k, j] * wvT[k, e]         (PE fp32r)
  PT[h][j, i]     = exp(biasT[h][j, i])              (ACT, bf16 in/out, in-place)
  sumexp[h][i]    = sum_j PT[h][j, i]                          (PE, ones-mat)
  OT[b][hd, i]    = (sum_j V[j,(b,hd)] * PT[h][j,i]) / sumexp  (PE bf16 + DVE)
  out[b][i, m]    = sum_hd OT[b][hd, i] * woT[hd, m] + c[m]    (PE fp32r + DVE)
  c[m]            = v_b @ woT + out_b   (v_b folded through: P rows sum to 1)

The attention matmuls run in bf16 (fp32r matmuls require dst start_partition
0 and even moving counts, which the col-tiled per-head layout can't satisfy);
the big projections run fp32r (11-bit-mantissa fp32) at full PE rate.
"""

import numpy as np

B, S, E = 16, 577, 1024
H, D = 16, 64
NCORES = 8
BPC = B // NCORES  # batches per core

# sequence tiles (partition-dim tiles of 128, last ragged 65)
STILES = [(0, 128), (128, 256), (256, 384), (384, 512), (512, 577)]
NST = len(STILES)
ICHUNKS = [(0, 289), (289, 577)]  # moving-dim chunks of the attention matmul
NKT = E // 128  # 8 contraction tiles
NEC = E // 512  # 2 free-dim chunks of the projections


def _build_program(debug=False):
    import concourse.bass as bass
    import concourse.bacc as bacc
    import concourse.mybir as mybir
    import concourse.tile as tile

    dt = mybir.dt
    f32 = dt.float32
    f32r = dt.float32r
    bf16 = dt.bfloat16
    Exp = mybir.ActivationFunctionType.Exp
    PSUM = bass.MemorySpace.PSUM

    nc = bacc.Bacc("TRN2", target_bir_lowering=False, debug=False, num_devices=NCORES)

    hT = nc.declare_dram_parameter("hiddenT", [BPC, E, S], f32r, isOutput=False)
    wvT = nc.declare_dram_parameter("wvT", [E, E], f32r, isOutput=False)
    woT = nc.declare_dram_parameter("woT", [E, E], f32r, isOutput=False)
    vb = nc.declare_dram_parameter("v_b", [E], f32r, isOutput=False)
    ob = nc.declare_dram_parameter("out_b", [E], f32, isOutput=False)
    bT = nc.declare_dram_parameter("biasT", [H, S, S], bf16, isOutput=False)
    out = nc.declare_dram_parameter("out", [BPC, S, E], f32, isOutput=True)
    if debug:
        dbg_v = nc.declare_dram_parameter("dbg_v", [NST, 128, BPC, E], bf16, isOutput=True)
        dbg_inv = nc.declare_dram_parameter("dbg_inv", [H // 2, 128, S], f32, isOutput=True)
        dbg_ot = nc.declare_dram_parameter("dbg_ot", [BPC, NKT, 128, S], f32r, isOutput=True)

    with tile.TileContext(nc) as tc:
        with (
            tc.tile_pool(name="const", bufs=1) as const_pool,
            tc.tile_pool(name="wop", bufs=1) as wo_pool,
            tc.tile_pool(name="vsb", bufs=NST) as v_pool,
            tc.tile_pool(name="ptp", bufs=5) as pt_pool,
            tc.tile_pool(name="invp", bufs=4) as invs_pool,
            tc.tile_pool(name="i2p", bufs=2) as i2_pool,
            tc.tile_pool(name="mvp", bufs=2) as mv_pool,
            tc.tile_pool(name="osbp", bufs=2) as osb_pool,
            tc.tile_pool(name="psum", bufs=2, space=PSUM) as psum_pool,
        ):
            # ---- constants + weight loads ---------------------------------
            ones_mat = const_pool.tile([128, 64], bf16, tag="ones", name="ones")
            nc.vector.memset(ones_mat[:], 1.0)

            v_sb = [
                v_pool.tile([128, BPC, E], bf16, tag="v", name="v") for _ in STILES
            ]

            # ---- phase A: V projection (fp32r) ----------------------------
            with (
                tc.tile_pool(name="wvp", bufs=1) as wv_pool,
                tc.tile_pool(name="htp", bufs=BPC) as ht_pool,
            ):
                # interleave wv/ht[0] per k-tile so the first V-proj
                # accumulation chain starts after ~2 tiles instead of 8.7MB
                wv_t = wv_pool.tile([128, NKT, E], f32r, tag="wv", name="wv")
                ht_t = [
                    ht_pool.tile([128, NKT, S], f32r, tag="ht", name="ht")
                    for _ in range(BPC)
                ]
                for kt in range(NKT):
                    nc.sync.dma_start(wv_t[:, kt, :], wvT[bass.ts(kt, 128), :])
                    nc.sync.dma_start(ht_t[0][:, kt, :], hT[0, bass.ts(kt, 128), :])
                for b in range(1, BPC):
                    for kt in range(NKT):
                        nc.sync.dma_start(ht_t[b][:, kt, :], hT[b, bass.ts(kt, 128), :])

                # bias DMAs + in-place exp: emitted early so the DMA queue
                # streams bias under the V projection; ACT is otherwise idle
                pt_t = {}
                for h in range(H):
                    p = pt_pool.tile([128, NST, S], bf16, tag="pt", name="pt")
                    nc.sync.dma_start(
                        p[:, 0:4, :],
                        bT[h, 0:512, :].rearrange("(jt p) i -> p jt i", p=128),
                    )
                    nc.sync.dma_start(p[0:65, 4, :], bT[h, 512:577, :])
                    nc.scalar.activation(p[:, 0:4, :], p[:, 0:4, :], Exp)
                    nc.scalar.activation(p[0:65, 4, :], p[0:65, 4, :], Exp)
                    pt_t[h] = p

                for b in range(BPC):
                    for st, (s0, s1) in enumerate(STILES):
                        ssz = s1 - s0
                        for ec in range(NEC):
                            ps = psum_pool.tile([128, 512], f32, tag="ps8", name="vps", bufs=6)
                            for kt in range(NKT):
                                nc.tensor.matmul(
                                    ps[0:ssz, :],
                                    ht_t[b][:, kt, s0:s1],
                                    wv_t[:, kt, bass.ts(ec, 512)],
                                    start=(kt == 0),
                                    stop=(kt == NKT - 1),
                                )
                            nc.vector.tensor_copy(
                                v_sb[st][0:ssz, b, bass.ts(ec, 512)], ps[0:ssz, :]
                            )

                if debug:
                    for st in range(NST):
                        nc.sync.dma_start(dbg_v[st], v_sb[st][:])

            # ---- phase B: per-head-pair softmax + attention (bf16) --------
            with tc.tile_pool(name="otp", bufs=BPC * NKT) as ot_pool:
                ot_t = {}
                for b in range(BPC):
                    for kt in range(NKT):
                        ot_t[b, kt] = ot_pool.tile([128, S], f32r, tag="ot", name="ot")

                wo_t = wo_pool.tile([128, NKT, E], f32r, tag="wo", name="wo")
                for kt in range(NKT):
                    nc.sync.dma_start(wo_t[:, kt, :], woT[bass.ts(kt, 128), :])
                vb_sb = const_pool.tile([128, NKT], f32r, tag="vb", name="vb")
                nc.sync.dma_start(vb_sb[:], vb.rearrange("(a p) -> p a", p=128))
                ob_sb = const_pool.tile([1, E], f32, tag="ob", name="ob")
                nc.sync.dma_start(ob_sb[:], ob.rearrange("(a e) -> a e", a=1))

                for kt in range(NKT):
                    h0, h1 = 2 * kt, 2 * kt + 1

                    # paired sumexp: both heads concurrently via col tiling
                    inv2 = i2_pool.tile([128, S], f32, tag="i2", name="i2")
                    for (i0, i1) in ICHUNKS:
                        isz = i1 - i0
                        sps = psum_pool.tile([128, 289], f32, tag="sps", name="sps", bufs=2)
                        for jt, (j0, j1) in enumerate(STILES):
                            jsz = j1 - j0
                            nc.tensor.matmul(
                                sps[0:64, 0:isz],
                                ones_mat[0:jsz, :],
                                pt_t[h0][0:jsz, jt, i0:i1],
                                start=(jt == 0),
                                stop=(jt == NST - 1),
                                tile_position=(0, 0),
                            )
                            nc.tensor.matmul(
                                sps[64:128, 0:isz],
                                ones_mat[0:jsz, :],
                                pt_t[h1][0:jsz, jt, i0:i1],
                                start=(jt == 0),
                                stop=(jt == NST - 1),
                                tile_position=(0, 64),
                            )
                        nc.vector.reciprocal(inv2[0:1, i0:i1], sps[0:1, 0:isz])
                        nc.vector.reciprocal(inv2[64:65, i0:i1], sps[64:65, 0:isz])

                    # odd head's row lives on partition 64; hop it to
                    # partition 0 via SBUF->SBUF DMA for the broadcast
                    mv = mv_pool.tile([1, S], f32, tag="mv", name="mv")
                    nc.scalar.dma_start(mv[0:1, :], inv2[64:65, :])
                    inv_bc0 = invs_pool.tile([128, S], f32, tag="invs", name="invs")
                    nc.gpsimd.partition_broadcast(inv_bc0[:], inv2[0:1, :])
                    inv_bc1 = invs_pool.tile([128, S], f32, tag="invs", name="invs")
                    nc.gpsimd.partition_broadcast(inv_bc1[:], mv[0:1, :])

                    if debug:
                        nc.sync.dma_start(dbg_inv[kt][0:64], inv_bc0[0:64, :])
                        nc.sync.dma_start(dbg_inv[kt][64:128], inv_bc1[64:128, :])

                    # attention: psum[(h%2)*64+d, i] over j tiles, col-tiled
                    for b in range(BPC):
                        for (i0, i1) in ICHUNKS:
                            isz = i1 - i0
                            ps = psum_pool.tile([128, 289], f32, tag="ps8", name="aps", bufs=6)
                            for jt, (j0, j1) in enumerate(STILES):
                                jsz = j1 - j0
                                nc.tensor.matmul(
                                    ps[0:64, 0:isz],
                                    v_sb[jt][0:jsz, b, h0 * 64 : h0 * 64 + 64],
                                    pt_t[h0][0:jsz, jt, i0:i1],
                                    start=(jt == 0),
                                    stop=(jt == NST - 1),
                                    tile_position=(0, 0),
                                )
                                nc.tensor.matmul(
                                    ps[64:128, 0:isz],
                                    v_sb[jt][0:jsz, b, h1 * 64 : h1 * 64 + 64],
                                    pt_t[h1][0:jsz, jt, i0:i1],
                                    start=(jt == 0),
                                    stop=(jt == NST - 1),
                                    tile_position=(0, 64),
                                )
                            nc.vector.tensor_mul(
                                ot_t[b, kt][0:64, i0:i1],
                                ps[0:64, 0:isz],
                                inv_bc0[0:64, i0:i1],
                            )
                            nc.vector.tensor_mul(
                                ot_t[b, kt][64:128, i0:i1],
                                ps[64:128, 0:isz],
                                inv_bc1[64:128, i0:i1],
                            )

                if debug:
                    for b in range(BPC):
                        for kt in range(NKT):
                            nc.sync.dma_start(dbg_ot[b, kt], ot_t[b, kt][:])

                # ---- c = v_b @ woT + out_b, broadcast -----------------
                c_sb = const_pool.tile([1, E], f32, tag="c", name="c")
                c_bc = const_pool.tile([128, E], f32, tag="cbc", name="cbc")
                for mc in range(NEC):
                    cps = psum_pool.tile([128, 512], f32, tag="ps8", name="cps", bufs=6)
                    for kt in range(NKT):
                        nc.tensor.matmul(
                            cps[0:1, :],
                            vb_sb[:, kt : kt + 1],
                            wo_t[:, kt, bass.ts(mc, 512)],
                            start=(kt == 0),
                            stop=(kt == NKT - 1),
                        )
                    nc.vector.tensor_add(
                        c_sb[:, bass.ts(mc, 512)], cps[0:1, :], ob_sb[:, bass.ts(mc, 512)]
                    )
                nc.gpsimd.partition_broadcast(c_bc[:], c_sb[:])

                # ---- phase C: output projection (fp32r) -------------------
                for b in range(BPC):
                    for (s0, s1) in STILES:
                        ssz = s1 - s0
                        for mc in range(NEC):
                            ps = psum_pool.tile([128, 512], f32, tag="ps8", name="ops", bufs=6)
                            for kt in range(NKT):
                                nc.tensor.matmul(
                                    ps[0:ssz, :],
                                    ot_t[b, kt][:, s0:s1],
                                    wo_t[:, kt, bass.ts(mc, 512)],
                                    start=(kt == 0),
                                    stop=(kt == NKT - 1),
                                )
                            osb = osb_pool.tile([128, 512], f32, tag="osb", name="osb")
                            nc.vector.tensor_add(
                                osb[0:ssz, :],
                                ps[0:ssz, :],
                                c_bc[0:ssz, bass.ts(mc, 512)],
                            )
                            nc.sync.dma_start(
                                out[b, s0:s1, bass.ts(mc, 512)], osb[0:ssz, :]
                            )

    nc.finalize()
    return nc


def _to_fp32r(a):
    """Round fp32 to the fp32r format: RNE to 11 explicit mantissa bits,
    low 12 bits of the word zeroed (matches walrus fp32_to_fp32r)."""
    u = np.ascontiguousarray(a, dtype=np.float32).view(np.uint32)
    r = (u.astype(np.uint64) + 0x7FF + ((u >> 12) & 1)).astype(np.uint32) & np.uint32(
        0xFFFFF000
    )
    return r.view(np.float32)


_NC_CACHE = None


def _get_program():
    global _NC_CACHE
    if _NC_CACHE is None:
        _NC_CACHE = _build_program()
    return _NC_CACHE


def kernel(
    hidden_states,
    q_w,
    q_b,
    v_w,
    v_b,
    out_w,
    out_b,
    share_key,
    share_bias,
    layer,
    _trace=False,
):
    """Full-input / full-output entry point. q_w/q_b/share_key/layer are
    mathematically irrelevant (softmax shift invariance) and unused."""
    from concourse.bass_utils import run_bass_kernel_spmd

    hidden_states = np.ascontiguousarray(np.asarray(hidden_states, dtype=np.float32))
    v_w = np.asarray(v_w, dtype=np.float32)
    v_b = np.ascontiguousarray(np.asarray(v_b, dtype=np.float32))
    out_w = np.asarray(out_w, dtype=np.float32)
    out_b = np.ascontiguousarray(np.asarray(out_b, dtype=np.float32))
    share_bias = np.asarray(share_bias, dtype=np.float32)

    # host-side layout transforms (transposes + fp32r rounding, no math).
    hiddenT = _to_fp32r(np.ascontiguousarray(hidden_states.transpose(0, 2, 1)))
    wvT = _to_fp32r(np.ascontiguousarray(v_w.T))  # [k, e]
    woT = _to_fp32r(np.ascontiguousarray(out_w.T))  # [k, m]
    v_b = _to_fp32r(v_b)
    import ml_dtypes

    biasT = np.ascontiguousarray(
        share_bias.transpose(0, 2, 1).astype(ml_dtypes.bfloat16)
    )  # [H, j, i] bf16

    nc = _get_program()
    in_maps = []
    for c in range(NCORES):
        in_maps.append(
            {
                "hiddenT": hiddenT[c * BPC : (c + 1) * BPC],
                "wvT": wvT,
                "woT": woT,
                "v_b": v_b,
                "out_b": out_b,
                "biasT": biasT,
            }
        )
    res = run_bass_kernel_spmd(nc, in_maps, list(range(NCORES)), trace=_trace)
    out = np.concatenate([res.results[c]["out"] for c in range(NCORES)], axis=0)
    if _trace:
        kernel.last_results = res
    return out

